# revision 11
# baseline (speedup 1.0000x reference)
"""Trainium2 Bass kernel for nn_MCPInitEmbedding (segment_reduce).

Problem: out[b, s, :] = sum_{j<100} (weights[b, idx[b,s,j]] * w + bias)
       = (sum_j weights[b, idx[b,s,j]]) * w + 100 * bias
So the kernel gathers-and-sums scalars per set (the segment reduce), then
expands rank-1 with the projection weights on the tensor engine
(K=2 matmul: [s_sums; 100]^T @ [w; b]).

Sharding: pure data parallel, 2 batches per core across 8 cores.
Gather: gpsimd ap_gather; each 16-partition group gathers its own
12512-slot index list (125 sets x 100 members, chunked + padded to
multiples of 16) from a per-partition-replicated weight table. Gathers
are chunked so the DVE segmented reduces pipeline underneath them; a
tiny warmup gather prefetches the Q7 library IRAM during the table DMA.

Measured on trn2: ~717 us/core, dominated by the ap_gather ucode rate
(~27 ns per index slot per 16-partition group; all 16 partitions of a
group share one index list, so the per-slot FIFO traffic is 16x4B).
"""
import numpy as np

import concourse.bacc as bacc
import concourse.tile as tile
import concourse.mybir as mybir
from concourse.bass_utils import run_bass_kernel_spmd

B = 16
N_ITEMS = 10000
N_SETS = 1000
SET_SZ = 100
D = 128
N_CORES = 8
BPC = B // N_CORES  # batches per core = 2

SETS_PER_GROUP = N_SETS // 8  # 125
CH_SETS = [32, 32, 32, 22, 7]  # sets per chunk per group
CH_NIDX = [((ns * SET_SZ + 15) // 16) * 16 for ns in CH_SETS]  # 3200,...,2912
NPG = sum(CH_NIDX) // 16  # 782 index columns per partition per batch

F32 = mybir.dt.float32
I16 = mybir.dt.int16

_CACHED = {}


def _build_program():
    nc = bacc.Bacc("TRN2", target_bir_lowering=False, debug=False,
                   num_devices=N_CORES)
    wt = nc.dram_tensor("wt", [128, BPC * N_ITEMS], F32,
                        kind="ExternalInput").ap()
    idx = nc.dram_tensor("idx", [128, BPC * NPG], I16,
                         kind="ExternalInput").ap()
    wbr = nc.dram_tensor("wbr", [128, 2 * D], F32, kind="ExternalInput").ap()
    out = nc.dram_tensor("out", [BPC, N_SETS, D], F32,
                         kind="ExternalOutput").ap()

    with tile.TileContext(nc) as tc:
        with (
            tc.tile_pool(name="main", bufs=1) as pool,
            tc.tile_pool(name="gp", bufs=2) as gpool,
        ):
            wtile = pool.tile([128, BPC * N_ITEMS], F32)
            itile = pool.tile([128, BPC * NPG], I16)
            wbt = pool.tile([128, 2 * D], F32)
            osb = pool.tile([128, SETS_PER_GROUP * D], F32)

            nc.sync.dma_start(itile[:], idx)
            nc.sync.dma_start(wbt[:], wbr)
            # tiny warmup gather: pays the ~6us Q7 library IRAM load while
            # the table DMA is still in flight
            warm = pool.tile([128, 16], F32)
            nc.vector.memset(warm[:, :], 0.0)
            widx = pool.tile([128, 1], I16)
            nc.vector.memset(widx[:, :], 0)
            nc.gpsimd.ap_gather(warm[:, :16], warm[:, :16], widx[:, :1],
                                128, 16, 1, 16)
            # per-batch table loads so the first gather starts early
            for bb in range(BPC):
                sl = slice(bb * N_ITEMS, (bb + 1) * N_ITEMS)
                nc.sync.dma_start(wtile[:, sl], wt[:, sl])

            for bb in range(BPC):
                red = gpool.tile([128, SETS_PER_GROUP], F32, tag="red")
                col0 = 0
                set0 = 0
                for ns, nidx in zip(CH_SETS, CH_NIDX):
                    slots = ns * SET_SZ
                    cols = nidx // 16
                    g = gpool.tile([128, max(CH_NIDX)], F32, tag="g")
                    nc.gpsimd.ap_gather(
                        g[:, :nidx],
                        wtile[:, bb * N_ITEMS : (bb + 1) * N_ITEMS],
                        itile[:, bb * NPG + col0 : bb * NPG + col0 + cols],
                        128, N_ITEMS, 1, nidx,
                    )
                    # segmented sum: runs of 100 -> per-group set sums
                    nc.vector.tensor_reduce(
                        red[:, set0 : set0 + ns],
                        g[:, :slots].rearrange("p (s j) -> p s j", j=SET_SZ),
                        axis=mybir.AxisListType.X,
                        op=mybir.AluOpType.add,
                    )
                    # rank-1 expansion + bias on DVE, per chunk:
                    # osb[p, s, :] = red[p, s] * w + 100*b  (8 group rows used)
                    seg = osb[:, set0 * D : (set0 + ns) * D].rearrange(
                        "p (s d) -> p s d", d=D)
                    red_b = red[:, set0 : set0 + ns].rearrange(
                        "p (s u) -> p s u", u=1).to_broadcast([128, ns, D])
                    w_b = wbt[:, 0:D].rearrange(
                        "p (s d) -> p s d", s=1).to_broadcast([128, ns, D])
                    b_b = wbt[:, D : 2 * D].rearrange(
                        "p (s d) -> p s d", s=1).to_broadcast([128, ns, D])
                    nc.vector.tensor_tensor(
                        out=seg, in0=red_b, in1=w_b, op=mybir.AluOpType.mult)
                    nc.vector.scalar_tensor_tensor(
                        out=seg, in0=b_b, scalar=float(SET_SZ), in1=seg,
                        op0=mybir.AluOpType.mult, op1=mybir.AluOpType.add)
                    # store this chunk: sets 125*g + [set0, set0+ns)
                    nc.sync.dma_start(
                        out[bb].rearrange("(G s) d -> G s d",
                                          s=SETS_PER_GROUP)[:, set0 : set0 + ns, :],
                        osb[::16, set0 * D : (set0 + ns) * D],
                    )
                    col0 += cols
                    set0 += ns

    nc.compile()
    return nc


def _wrap_indices(mem_core: np.ndarray) -> np.ndarray:
    """membership rows for one core [BPC, 1000, 100] int -> [128, BPC*NPG] i16.

    Per batch, per 16-partition group, per gather chunk: flatten the chunk's
    (set, member) indices, pad to a multiple of 16, and wrap so slot
    k = s*16 + p lives at [16*grp + p, col0 + s].
    """
    idx16 = np.zeros((128, BPC * NPG), dtype=np.int16)
    for bb in range(BPC):
        for grp in range(8):
            col0 = bb * NPG
            set0 = grp * SETS_PER_GROUP
            for ns, nidx in zip(CH_SETS, CH_NIDX):
                flat = mem_core[bb, set0 : set0 + ns, :].reshape(-1)
                pad = np.zeros(nidx, dtype=np.int16)
                pad[: flat.size] = flat.astype(np.int16)
                cols = nidx // 16
                idx16[16 * grp : 16 * grp + 16, col0 : col0 + cols] = (
                    pad.reshape(cols, 16).T
                )
                col0 += cols
                set0 += ns
    return idx16


def kernel(weights, membership, w, b):
    weights = np.asarray(weights, dtype=np.float32)
    membership = np.asarray(membership)
    w = np.asarray(w, dtype=np.float32)
    b = np.asarray(b, dtype=np.float32)

    if "nc" not in _CACHED:
        _CACHED["nc"] = _build_program()
    nc = _CACHED["nc"]

    wbr_np = np.ascontiguousarray(np.broadcast_to(
        np.concatenate([w, b]).astype(np.float32)[None, :], (128, 2 * D)))
    in_maps = []
    for c in range(N_CORES):
        wt_np = np.ascontiguousarray(
            np.broadcast_to(
                weights[c * BPC : (c + 1) * BPC].reshape(1, BPC * N_ITEMS),
                (128, BPC * N_ITEMS),
            )
        )
        idx_np = _wrap_indices(membership[c * BPC : (c + 1) * BPC])
        in_maps.append({"wt": wt_np, "idx": idx_np, "wbr": wbr_np})

    res = run_bass_kernel_spmd(nc, in_maps, core_ids=list(range(N_CORES)))
    out = np.concatenate(
        [res.results[c]["out"] for c in range(N_CORES)], axis=0
    )
    return out.astype(np.float32)


# revision 12
# speedup vs baseline: 1.0186x; 1.0186x over previous
"""Trainium2 Bass kernel for nn_MCPInitEmbedding (segment_reduce).

Problem: out[b, s, :] = sum_{j<100} (weights[b, idx[b,s,j]] * w + bias)
       = (sum_j weights[b, idx[b,s,j]]) * w + 100 * bias
So the kernel gathers-and-sums scalars per set (the segment reduce), then
expands rank-1 with the projection weights on the tensor engine
(K=2 matmul: [s_sums; 100]^T @ [w; b]).

Sharding: pure data parallel, 2 batches per core across 8 cores.
Gather: gpsimd ap_gather; each 16-partition group gathers its own
12512-slot index list (125 sets x 100 members, chunked + padded to
multiples of 16) from a per-partition-replicated weight table. Gathers
are chunked so the DVE segmented reduces pipeline underneath them; a
tiny warmup gather prefetches the Q7 library IRAM during the table DMA.

Measured on trn2: ~717 us/core, dominated by the ap_gather ucode rate
(~27 ns per index slot per 16-partition group; all 16 partitions of a
group share one index list, so the per-slot FIFO traffic is 16x4B).
"""
import numpy as np

import concourse.bacc as bacc
import concourse.tile as tile
import concourse.mybir as mybir
from concourse.bass_utils import run_bass_kernel_spmd

B = 16
N_ITEMS = 10000
N_SETS = 1000
SET_SZ = 100
D = 128
N_CORES = 8
BPC = B // N_CORES  # batches per core = 2

SETS_PER_GROUP = N_SETS // 8  # 125
CH_SETS = [32, 32, 32, 22, 7]  # sets per chunk per group
CH_NIDX = [((ns * SET_SZ + 15) // 16) * 16 for ns in CH_SETS]  # 3200,...,2912
NPG = sum(CH_NIDX) // 16  # 782 index columns per partition per batch

F32 = mybir.dt.float32
I16 = mybir.dt.int16

_CACHED = {}


def _build_program():
    nc = bacc.Bacc("TRN2", target_bir_lowering=False, debug=False,
                   num_devices=N_CORES)
    wt = nc.dram_tensor("wt", [128, BPC * N_ITEMS], F32,
                        kind="ExternalInput").ap()
    idx = nc.dram_tensor("idx", [128, BPC * NPG], I16,
                         kind="ExternalInput").ap()
    wb = nc.dram_tensor("wb", [2, D], F32, kind="ExternalInput").ap()
    out = nc.dram_tensor("out", [BPC, N_SETS, D], F32,
                         kind="ExternalOutput").ap()

    with tile.TileContext(nc) as tc:
        with (
            tc.tile_pool(name="main", bufs=1) as pool,
            tc.tile_pool(name="gp", bufs=2) as gpool,
            tc.tile_pool(name="ps", bufs=2, space="PSUM") as psp,
        ):
            wtile = pool.tile([128, BPC * N_ITEMS], F32)
            itile = pool.tile([128, BPC * NPG], I16)
            wbt = pool.tile([2, D], F32)

            # batch-0 table first: it gates the first gather
            nc.sync.dma_start(wtile[:, :N_ITEMS], wt[:, :N_ITEMS])
            nc.sync.dma_start(itile[:, :NPG], idx[:, :NPG])
            nc.sync.dma_start(wbt[:], wb)
            # tiny warmup gather: pays the ~6us Q7 library IRAM load while
            # the table DMA is still in flight
            warm = pool.tile([128, 16], F32)
            nc.vector.memset(warm[:, :], 0.0)
            widx = pool.tile([128, 1], I16)
            nc.vector.memset(widx[:, :], 0)
            nc.gpsimd.ap_gather(warm[:, :16], warm[:, :16], widx[:, :1],
                                128, 16, 1, 16)
            nc.sync.dma_start(wtile[:, N_ITEMS:], wt[:, N_ITEMS:])
            nc.sync.dma_start(itile[:, NPG:], idx[:, NPG:])

            for bb in range(BPC):
                red = gpool.tile([128, SETS_PER_GROUP], F32, tag="red")
                col0 = 0
                set0 = 0
                for ns, nidx in zip(CH_SETS, CH_NIDX):
                    slots = ns * SET_SZ
                    cols = nidx // 16
                    g = gpool.tile([128, max(CH_NIDX)], F32, tag="g")
                    nc.gpsimd.ap_gather(
                        g[:, :nidx],
                        wtile[:, bb * N_ITEMS : (bb + 1) * N_ITEMS],
                        itile[:, bb * NPG + col0 : bb * NPG + col0 + cols],
                        128, N_ITEMS, 1, nidx,
                    )
                    # segmented sum: runs of 100 -> per-group set sums
                    nc.vector.tensor_reduce(
                        red[:, set0 : set0 + ns],
                        g[:, :slots].rearrange("p (s j) -> p s j", j=SET_SZ),
                        axis=mybir.AxisListType.X,
                        op=mybir.AluOpType.add,
                    )
                    col0 += cols
                    set0 += ns

                # compact the 8 group rows (partitions 0,16,..,112) into one
                # row of 1000 set sums; row 1 stays SET_SZ so the K=2 matmul
                # adds SET_SZ*b
                srow = gpool.tile([2, 1024], F32, tag="srow")
                nc.vector.memset(srow[0:2, :], float(SET_SZ))
                nc.sync.dma_start(
                    srow[0:1, 0:N_SETS].rearrange("p (g s) -> p g s",
                                                  s=SETS_PER_GROUP),
                    red[::16, :],
                )
                # rank-1 expansion + bias: out[set, :] = s*w + 100*b
                ob = gpool.tile([128, 1024], F32, tag="ob")
                for m in range(8):
                    ps = psp.tile([128, D], F32, tag="ps")
                    nc.tensor.matmul(
                        out=ps[:],
                        lhsT=srow[0:2, m * 128 : (m + 1) * 128],
                        rhs=wbt[:],
                        start=True,
                        stop=True,
                    )
                    nc.vector.tensor_copy(ob[:, m * D : (m + 1) * D], ps[:])
                # store: sets = m*128 + p
                nc.sync.dma_start(
                    out[bb, : 7 * 128, :].rearrange("(m p) d -> p m d", p=128),
                    ob[:, : 7 * D].rearrange("p (m d) -> p m d", d=D),
                )
                nc.sync.dma_start(
                    out[bb, 7 * 128 : N_SETS, :],
                    ob[: N_SETS - 7 * 128, 7 * D : 8 * D],
                )

    nc.compile()
    return nc


def _wrap_indices(mem_core: np.ndarray) -> np.ndarray:
    """membership rows for one core [BPC, 1000, 100] int -> [128, BPC*NPG] i16.

    Per batch, per 16-partition group, per gather chunk: flatten the chunk's
    (set, member) indices, pad to a multiple of 16, and wrap so slot
    k = s*16 + p lives at [16*grp + p, col0 + s].
    """
    idx16 = np.zeros((128, BPC * NPG), dtype=np.int16)
    for bb in range(BPC):
        for grp in range(8):
            col0 = bb * NPG
            set0 = grp * SETS_PER_GROUP
            for ns, nidx in zip(CH_SETS, CH_NIDX):
                flat = mem_core[bb, set0 : set0 + ns, :].reshape(-1)
                pad = np.zeros(nidx, dtype=np.int16)
                pad[: flat.size] = flat.astype(np.int16)
                cols = nidx // 16
                idx16[16 * grp : 16 * grp + 16, col0 : col0 + cols] = (
                    pad.reshape(cols, 16).T
                )
                col0 += cols
                set0 += ns
    return idx16


def kernel(weights, membership, w, b):
    weights = np.asarray(weights, dtype=np.float32)
    membership = np.asarray(membership)
    w = np.asarray(w, dtype=np.float32)
    b = np.asarray(b, dtype=np.float32)

    if "nc" not in _CACHED:
        _CACHED["nc"] = _build_program()
    nc = _CACHED["nc"]

    wb_np = np.stack([w, b]).astype(np.float32)  # [2, 128]
    in_maps = []
    for c in range(N_CORES):
        wt_np = np.ascontiguousarray(
            np.broadcast_to(
                weights[c * BPC : (c + 1) * BPC].reshape(1, BPC * N_ITEMS),
                (128, BPC * N_ITEMS),
            )
        )
        idx_np = _wrap_indices(membership[c * BPC : (c + 1) * BPC])
        in_maps.append({"wt": wt_np, "idx": idx_np, "wb": wb_np})

    res = run_bass_kernel_spmd(nc, in_maps, core_ids=list(range(N_CORES)))
    out = np.concatenate(
        [res.results[c]["out"] for c in range(N_CORES)], axis=0
    )
    return out.astype(np.float32)
